# revision 14
# baseline (speedup 1.0000x reference)
"""Disentangled attention (fused common+personal QKV + MHA) on 8 TRN2 cores.

Data-parallel over batch N=8 (one batch element per core, no communication).
Host pre-sums W_c+W_p / b_c+b_p exactly in f32, casts to bf16, pre-transposes x.

v4 (from v3 trace analysis: EXP stream was already gapless at 64.7us, but
prologue was 17.9us and epilogue 22.4us):
  - ACT saturation core unchanged: energies flow through two ping-ponged
    3-bank PSUM slabs, each drained by ONE N<=1536 exp; units c-major.
  - output is stored TRANSPOSED ([D, S] DRAM, host un-transposes): the
    attn@V result is produced d-on-partitions, so no DMA transposes at all.
  - softmax normalization without transposing denominators: dq rows 32t hold
    sum_k exp for head t; DVE reciprocal -> rq rows; a K=1 ones-matmul
    broadcasts row 32t across a head's 64 partitions (rbc, PSUM); the final
    out = outU(bf16, unnormalized evac) * rbc on DVE. No dn evac, no
    dn transpose, no per-head tiny reciprocals.
  - chains evacuate unnormalized immediately (EU) so the 2 chain PSUM banks
    (tags ao/dq) recycle fast; 6 slab banks + 2 chain banks = 8.
  - prologue: input DMAs are the first instructions (crit-first: xT c0-half,
    biases, wq0/wk0); warmup is 6 chained f32 matmuls on the tiny bv tile
    (lands early) instead of 10 N=512 matmuls that blocked the projections.
  - scalar queue carries ONLY the 44 exps (stores/copies live on sync/
    gpsimd/vector) so nothing ever stalls the act stream.
"""

import os
from contextlib import ExitStack

import numpy as np
import ml_dtypes

import concourse.bass as bass
import concourse.tile as tile
import concourse.mybir as mybir
from concourse import bacc
from concourse.bass_utils import run_bass_kernel_spmd

B, S, D, H, HD = 8, 1024, 512, 8, 64
P = 128
KB = D // P           # 4 contraction blocks
SB = S // P           # 8 sequence tiles
NU = 4 * SB           # 32 energy units per head pair
SU = 3                # units per PSUM slab (3 banks)
NS = (NU + SU - 1) // SU
BF16 = mybir.dt.bfloat16
F32 = mybir.dt.float32
SCALE = 1.0 / float(np.sqrt(D))

NPBF16 = ml_dtypes.bfloat16


def _unit(j, c, h01):
    return c * 16 + 2 * j + h01


def _bcast_ap(ap, parts):
    """Broadcast a [1, ...] AP across `parts` partitions (stride-0 partition dim)."""
    return bass.AP(tensor=ap.tensor, offset=ap.offset, ap=[[0, parts]] + list(ap.ap[1:]))


def emit_kernel(ctx: ExitStack, tc: tile.TileContext):
    nc = tc.nc

    xT_d = nc.dram_tensor("xT", [P, KB, S], BF16, kind="ExternalInput")
    wq_d = nc.dram_tensor("wq", [P, KB, KB, P], BF16, kind="ExternalInput")  # [p, k, b, col]
    wk_d = nc.dram_tensor("wk", [P, KB, KB, P], BF16, kind="ExternalInput")
    wv_d = nc.dram_tensor("wv", [P, KB, D], BF16, kind="ExternalInput")
    bq_d = nc.dram_tensor("bq", [P, KB], F32, kind="ExternalInput")
    bk_d = nc.dram_tensor("bk", [P, KB], F32, kind="ExternalInput")
    bv_d = nc.dram_tensor("bv", [1, D], F32, kind="ExternalInput")
    on32_d = nc.dram_tensor("on32", [P, 32], BF16, kind="ExternalInput")
    bsel_d = nc.dram_tensor("bsel", [P, P], BF16, kind="ExternalInput")
    bself_d = nc.dram_tensor("bself", [P, P], F32, kind="ExternalInput")
    onesT_d = nc.dram_tensor("onesT", [P, 64], F32, kind="ExternalInput")
    out_d = nc.dram_tensor("out", [D, S], BF16, kind="ExternalOutput")  # TRANSPOSED

    consts = ctx.enter_context(tc.tile_pool(name="consts", bufs=1))
    persist = ctx.enter_context(tc.tile_pool(name="persist", bufs=1))

    xT_sb = persist.tile([P, KB, S], BF16, tag="xT", name="xT")
    wqb = [persist.tile([P, KB, P], BF16, tag=f"wq{b}", name=f"wq{b}") for b in range(KB)]
    wkb = [persist.tile([P, KB, P], BF16, tag=f"wk{b}", name=f"wk{b}") for b in range(KB)]
    wv_sb = persist.tile([P, KB, D], BF16, tag="wv", name="wv")
    bq_sb = consts.tile([P, KB], F32, tag="bq", name="bq")
    bk_sb = consts.tile([P, KB], F32, tag="bk", name="bk")
    bv_sb = consts.tile([P, D], F32, tag="bv", name="bv")
    on32 = consts.tile([P, 32], BF16, tag="on32", name="on32")
    bsel = consts.tile([P, P], BF16, tag="bsel", name="bsel")
    bself = consts.tile([P, P], F32, tag="bself", name="bself")
    onesT = consts.tile([P, 64], F32, tag="onesT", name="onesT")
    gate = consts.tile([1, 1], BF16, tag="gate", name="gate")

    # ---- input DMAs: wave 1 = critical prefix (xT c0-half, wq0/wk0,
    # biases, warmup ones); wave 2 (everything else) sits behind a gate op
    # that waits for wq0, so wave-1 gets the full HBM bandwidth.
    nc.sync.dma_start(out=onesT[:], in_=onesT_d[:])
    nc.sync.dma_start(out=xT_sb[:, 0:2, 0:512], in_=xT_d[:, 0:2, 0:512])
    nc.scalar.dma_start(out=wqb[0][:], in_=wq_d[:, :, 0, :])
    nc.scalar.dma_start(out=xT_sb[:, 2:4, 0:512], in_=xT_d[:, 2:4, 0:512])
    nc.gpsimd.dma_start(out=wkb[0][:], in_=wk_d[:, :, 0, :])
    nc.sync.dma_start(out=bq_sb[:], in_=bq_d[:])
    nc.sync.dma_start(out=bk_sb[:], in_=bk_d[:])
    # gate: a no-op gpsimd copy that depends on wq0 having landed
    nc.gpsimd.tensor_copy(out=gate[:], in_=wqb[0][0:1, 0:1, 0:1])
    nc.gpsimd.dma_start(out=xT_sb[:, :, 512:1024], in_=xT_d[:, :, 512:1024])
    nc.gpsimd.dma_start(out=wv_sb[:, 0:2, :], in_=wv_d[:, 0:2, :])
    nc.gpsimd.dma_start(out=wqb[1][:], in_=wq_d[:, :, 1, :])
    nc.gpsimd.dma_start(out=wkb[1][:], in_=wk_d[:, :, 1, :])
    nc.gpsimd.dma_start(out=wv_sb[:, 2:4, :], in_=wv_d[:, 2:4, :])
    nc.gpsimd.dma_start(out=wqb[2][:], in_=wq_d[:, :, 2, :])
    nc.gpsimd.dma_start(out=wkb[2][:], in_=wk_d[:, :, 2, :])
    nc.gpsimd.dma_start(out=wqb[3][:], in_=wq_d[:, :, 3, :])
    nc.gpsimd.dma_start(out=wkb[3][:], in_=wk_d[:, :, 3, :])
    nc.gpsimd.dma_start(out=on32[:], in_=on32_d[:])
    nc.gpsimd.dma_start(out=bsel[:], in_=bsel_d[:])
    nc.gpsimd.dma_start(out=bself[:], in_=bself_d[:])
    nc.gpsimd.dma_start(out=bv_sb[:], in_=_bcast_ap(bv_d[:], P))

    qT_sb = [persist.tile([P, S], BF16, tag=f"qT{b}", name=f"qT{b}") for b in range(KB)]
    kT_sb = [persist.tile([P, S], BF16, tag=f"kT{b}", name=f"kT{b}") for b in range(KB)]
    v64_sb = [persist.tile([P, H, HD], BF16, tag=f"v64_{j}", name=f"v64_{j}") for j in range(SB)]

    ptpool = ctx.enter_context(tc.tile_pool(name="ptpool", bufs=3))
    outpool = ctx.enter_context(tc.tile_pool(name="outpool", bufs=1))
    upool = ctx.enter_context(tc.tile_pool(name="upool", bufs=3))
    rqpool = ctx.enter_context(tc.tile_pool(name="rqpool", bufs=1))
    # PSUM: 2x 3-bank slabs (ping-pong) + 2 chain banks (ao/dq) = 8 banks
    ppsum = ctx.enter_context(tc.tile_pool(name="ppsum", bufs=1, space="PSUM"))

    outTp = [outpool.tile([P, S], BF16, tag=f"outTp{hp}", name=f"outTp{hp}")
             for hp in range(4)]
    rq_sb = {(q, c): rqpool.tile([P, 512], F32, tag=f"rq{q}{c}", name=f"rq{q}_{c}")
             for q in range(2) for c in range(2)}
    rqb_sb = {(q, c): rqpool.tile([P, 512], BF16, tag=f"rqb{q}{c}", name=f"rqb{q}_{c}")
              for q in range(2) for c in range(2)}

    # ---- emission primitives -------------------------------------------
    def proj_qk_quarter(b, t, c):
        """one (dout-block, {q|k}, half): 4-matmul k-chain + DVE bias evac"""
        wb, b_sb, dst = ((wqb, bq_sb, qT_sb), (wkb, bk_sb, kT_sb))[t]
        bank = ("ao", "dq")[(b + t + c) % 2]
        ps = ppsum.tile([P, 512], F32, tag=bank, name=f"pj{b}_{t}_{c}")
        for k in range(KB):
            nc.tensor.matmul(
                ps[:],
                wb[b][:, k, :],
                xT_sb[:, k, c * 512:(c + 1) * 512],
                start=(k == 0), stop=(k == KB - 1),
            )
        nc.vector.tensor_scalar_add(
            out=dst[b][:, c * 512:(c + 1) * 512],
            in0=ps[:],
            scalar1=b_sb[:, b:b + 1],
        )

    def proj_v_block(j):
        bank = ("ao", "dq")[j % 2]
        pv = ppsum.tile([P, 512], F32, tag=bank, name=f"pv{j}")
        for k in range(KB):
            nc.tensor.matmul(
                pv[:],
                xT_sb[:, k, j * P:(j + 1) * P],
                wv_sb[:, k, :],
                start=(k == 0), stop=(k == KB - 1),
            )
        nc.vector.tensor_add(
            out=v64_sb[j][:],
            in0=pv[:].rearrange("p (h d) -> p h d", h=H),
            in1=bv_sb[:].rearrange("p (h d) -> p h d", h=H),
        )

    def energy_slab(hp, n, ptf):
        """matmuls for units 3n..3n+2 into a 3-bank slab, then ONE exp."""
        units = list(range(SU * n, min(SU * n + SU, NU)))
        slab = ppsum.tile([P, SU, 512], F32, tag="slab", name=f"slab{hp}_{n}", bufs=2)
        for i, u in enumerate(units):
            c, j, h01 = u // 16, (u % 16) // 2, u % 2
            rows = slice(h01 * 64, h01 * 64 + 64)
            nc.tensor.matmul(
                slab[:, i, :],
                kT_sb[hp][rows, j * P:(j + 1) * P],
                qT_sb[hp][rows, c * 512:(c + 1) * 512],
                start=True, stop=True,
                tile_position=(h01 * 64, 0),
            )
        nc.scalar.activation(
            out=ptf[:, units[0]:units[-1] + 1, :],
            in_=slab[:, 0:len(units), :],
            func=mybir.ActivationFunctionType.Exp,
            scale=SCALE,
        )

    aobox = {}

    def attn_pair_half(hp, c, jlo, jhi):
        """j-range of a column-tiled attn@V pair chain (heads 2hp / 2hp+1 on
        PE cols 0-63 / 64-127 concurrently)."""
        if jlo == 0:
            aobox[(hp, c)] = ppsum.tile([P, 512], F32, tag="ao", name=f"ao{hp}_{c}")
        ao = aobox[(hp, c)]
        for j in range(jlo, jhi):
            for h01 in range(2):
                nc.tensor.matmul(
                    ao[h01 * 64:(h01 + 1) * 64, :],
                    v64_sb[j][:, 2 * hp + h01, :],
                    pts[hp][:, _unit(j, c, h01), :],
                    start=(j == 0), stop=(j == SB - 1),
                    tile_position=(0, h01 * 64),
                    skip_group_check=True,
                )

    def evac_u(hp, c, eng=None):
        """unnormalized bf16 evac of the attn chain -> frees the ao bank"""
        u = upool.tile([P, 512], BF16, tag="outU", name=f"oU{hp}_{c}")
        eng = eng or nc.vector
        if eng is nc.scalar:
            eng.copy(out=u[:], in_=aobox[(hp, c)][:])
        else:
            eng.tensor_copy(out=u[:], in_=aobox[(hp, c)][:])
        ubox[(hp, c)] = u

    dqbox = {}

    def denom_half(q, c, jlo, jhi):
        """j-range of the 4-way column-tiled ones-matmul denominator chain for
        head-quad q; sum_k exp lands in dq rows {0,32,64,96}."""
        if jlo == 0:
            dqbox[(q, c)] = ppsum.tile([P, 512], F32, tag="dq", name=f"dq{q}_{c}")
        dq = dqbox[(q, c)]
        for j in range(jlo, jhi):
            for t in range(4):
                hp, h01 = 2 * q + t // 2, t % 2
                nc.tensor.matmul(
                    dq[32 * t:32 * t + 32, :],
                    on32[:],
                    pts[hp][:, _unit(j, c, h01), :],
                    start=(j == 0), stop=(j == SB - 1),
                    tile_position=(0, 32 * t),
                    skip_group_check=True,
                )

    def recip(q, c, cast=True):
        """rq rows {0,32,64,96} = 1/denominator (fast approx, ~18 bits);
        cast to bf16 so the K=1 broadcast matmuls take the fast weight path.
        Frees the dq bank."""
        nc.vector.reciprocal_approx_fast(out=rq_sb[(q, c)][:], in_=dqbox[(q, c)][:])
        if cast:
            nc.vector.tensor_copy(out=rqb_sb[(q, c)][:], in_=rq_sb[(q, c)][:])

    def bcast_norm(hp, c, f32_rhs=False, bank="dq"):
        """K=1 ones-matmuls broadcast recip-den row 32t across each head's 64
        partitions (rbc), then out = outU * rbc -> normalized out^T tile.
        f32_rhs reads rq directly (no bf16 cast needed; used in the epilogue
        where latency beats PE cost)."""
        q = hp // 2
        sel_src = bself if f32_rhs else bsel
        r_src = rq_sb if f32_rhs else rqb_sb
        rbc = ppsum.tile([P, 512], F32, tag=bank, name=f"rbc{hp}_{c}")
        # one K=33 matmul: sel rows {32t0, 32t0+32} map the two heads'
        # reciprocal-den rows onto cols 0:64 / 64:128 (all output partitions)
        t0 = 2 * (hp % 2)
        nc.tensor.matmul(
            rbc[:],
            sel_src[32 * t0:32 * t0 + 33, :],
            r_src[(q, c)][32 * t0:32 * t0 + 33, :],
            start=True, stop=True,
        )
        nc.vector.tensor_mul(
            out=outTp[hp][:, c * 512:(c + 1) * 512],
            in0=ubox[(hp, c)][:],
            in1=rbc[:],
        )

    def store(hp, c, eng):
        eng.dma_start(
            out=out_d[hp * P:(hp + 1) * P, c * 512:(c + 1) * 512],
            in_=outTp[hp][:, c * 512:(c + 1) * 512],
        )

    # ---- prologue ------------------------------------------------------
    # HAM warmup: 14 chained f32 N=64 matmuls on the early-landing onesT
    # tile (f32 runs 4x slower per column = more busy-time per byte loaded).
    warm = ppsum.tile([P, 512], F32, tag="ao", name="warm")
    for w in range(16):
        nc.tensor.matmul(warm[0:64, 0:64], onesT[:], onesT[:],
                         start=(w == 0), stop=(w == 15))

    proj_qk_quarter(0, 0, 0)   # qT[0] c0   (ao bank: waits warm's rotation)
    for half in range(2):      # kT[0] j0-1 then j2-3 (N=256 sub-chains)
        sl = slice(half * 256, half * 256 + 256)
        pk = ppsum.tile([P, 256], F32, tag=("dq", "ao")[half], name=f"pk{half}")
        for k in range(KB):
            nc.tensor.matmul(pk[:], wkb[0][:, k, :], xT_sb[:, k, sl],
                             start=(k == 0), stop=(k == KB - 1))
        nc.vector.tensor_scalar_add(out=kT_sb[0][:, sl], in0=pk[:],
                                    scalar1=bk_sb[:, 0:1])

    pts = [None] * 4
    ubox = {}

    def A(hp, c, half):
        jlo, jhi = (0, 4) if half == 0 else (4, 8)
        return lambda: attn_pair_half(hp, c, jlo, jhi)

    def A2(hp, c, jlo, jhi):
        return lambda: attn_pair_half(hp, c, jlo, jhi)

    def EU(hp, c):
        return lambda: evac_u(hp, c)

    def Q(b, t, c):
        return lambda: proj_qk_quarter(b, t, c)

    def V(j):
        return lambda: proj_v_block(j)

    def DN(q, c, half):
        jlo, jhi = (0, 4) if half == 0 else (4, 8)
        return lambda: denom_half(q, c, jlo, jhi)

    def DN2(q, c, jlo, jhi):
        return lambda: denom_half(q, c, jlo, jhi)

    def R(q, c):
        return lambda: recip(q, c)

    def BN(hp, c):
        return lambda: bcast_norm(hp, c)

    def C(*fns):
        return lambda: [f() for f in fns]

    def ST(hp, c, eng):
        return lambda: store(hp, c, {"sync": nc.sync, "gpsimd": nc.gpsimd}[eng])

    # filler schedule: entry g of round hp runs after act g; it may only
    # consume exps from acts <= g of that round. None = slab-only slot.
    fill = {
        0: [Q(0, 0, 1), Q(0, 1, 1), Q(1, 0, 0), Q(1, 0, 1), Q(1, 1, 0), Q(1, 1, 1),
            V(0), V(1), V(2), V(3), V(4)],
        1: [V(5), V(6), V(7), Q(2, 0, 0), Q(2, 0, 1), Q(2, 1, 0), Q(2, 1, 1),
            A(0, 0, 0), C(A(0, 0, 1), EU(0, 0)),
            DN(0, 0, 0), DN(0, 0, 1),
            C(R(0, 0), BN(0, 0), ST(0, 0, "sync"))],
        2: [A(0, 1, 0), C(A(0, 1, 1), EU(0, 1)),
            DN(0, 1, 0), DN(0, 1, 1),
            C(R(0, 1), BN(0, 1), ST(0, 1, "gpsimd")),
            Q(3, 0, 0), Q(3, 0, 1), Q(3, 1, 0), Q(3, 1, 1),
            A(1, 0, 0), A(1, 0, 1)],
        3: [C(EU(1, 0), BN(1, 0), ST(1, 0, "sync"), A(1, 1, 0), A(1, 1, 1),
              EU(1, 1), BN(1, 1), ST(1, 1, "gpsimd")),
            A(2, 0, 0), C(A(2, 0, 1), EU(2, 0)),
            DN(1, 0, 0),
            A(2, 1, 0), C(A(2, 1, 1), EU(2, 1)),
            DN(1, 0, 1),
            C(R(1, 0), BN(2, 0), ST(2, 0, "sync"), A(3, 0, 0)),
            C(A(3, 0, 1), EU(3, 0), BN(3, 0), ST(3, 0, "gpsimd")),
            DN(1, 1, 0),
            A(3, 1, 0),
            C(A2(3, 1, 4, 6), DN2(1, 1, 4, 6)),
            C(A2(3, 1, 6, 7), DN2(1, 1, 6, 7)),
            C(A2(3, 1, 7, 8), DN2(1, 1, 7, 8))],
    }

    for hp in range(4):
        pts[hp] = ptpool.tile([P, NU, 512], BF16, tag="pt", name=f"pt{hp}")
        fl = fill[hp]
        fi = 0
        for n in range(NS):
            energy_slab(hp, n, pts[hp])
            while fi < len(fl) and fi <= n:
                if fl[fi] is not None:
                    fl[fi]()
                fi += 1
        while fi < len(fl):
            if fl[fi] is not None:
                fl[fi]()
            fi += 1

    # ---- epilogue: the only work allowed after the last exp -------------
    # DVE does recip + the two normalize muls; the (idle) Scalar engine does
    # the final evac and the rqb cast in parallel. outU(3,1) recycles
    # outU(2,1)'s buffer, but its reader BN(2,1).mul is on DVE while the
    # evac is on Scalar, so the WAR resolves via cross-queue sem (no FIFO
    # deadlock).
    recip(1, 1, cast=False)
    evac_u(3, 1, eng=nc.scalar)
    bcast_norm(2, 1, f32_rhs=True)
    store(2, 1, nc.sync)
    bcast_norm(3, 1, f32_rhs=True, bank="ao")
    store(3, 1, nc.sync)


_NC_CACHE = {}


def build_nc():
    if "nc" in _NC_CACHE:
        return _NC_CACHE["nc"]
    nc = bacc.Bacc("TRN2", target_bir_lowering=False, debug=False, num_devices=8)
    with tile.TileContext(nc) as tc:
        with ExitStack() as ctx:
            emit_kernel(ctx, tc)
    nc.compile()
    _NC_CACHE["nc"] = nc
    return nc


def host_prep(x, W_cq, b_cq, W_ck, b_ck, W_cv, b_cv, W_pq, b_pq, W_pk, b_pk, W_pv, b_pv):
    """Host-side sharding: exact f32 weight/bias fusion, bf16 casts, x transpose."""
    def blockw_bk(a, b2):
        # [din, dout] -> [p, k, b, col] (din = k*128+p, dout = b*128+col)
        w = (np.asarray(a, np.float32) + np.asarray(b2, np.float32)).astype(NPBF16)
        return np.ascontiguousarray(w.reshape(KB, P, KB, P).transpose(1, 0, 2, 3))

    def blockw_k(a, b2):
        w = (np.asarray(a, np.float32) + np.asarray(b2, np.float32)).astype(NPBF16)
        return np.ascontiguousarray(w.reshape(KB, P, D).transpose(1, 0, 2))

    wq = blockw_bk(W_cq, W_pq)
    wk = blockw_bk(W_ck, W_pk)
    wv = blockw_k(W_cv, W_pv)
    bq = (np.asarray(b_cq, np.float32) + np.asarray(b_pq, np.float32)).reshape(KB, P).T.copy()
    bk = (np.asarray(b_ck, np.float32) + np.asarray(b_pk, np.float32)).reshape(KB, P).T.copy()
    bv = (np.asarray(b_cv, np.float32) + np.asarray(b_pv, np.float32)).reshape(1, D).copy()
    on32 = np.ones((P, 32), dtype=NPBF16)
    bself = np.zeros((P, P), dtype=np.float32)
    for t0 in (0, 2):
        bself[32 * t0, 0:64] = 1
        bself[32 * t0 + 32, 64:128] = 1
    bsel = bself.astype(NPBF16)
    onesT = np.ones((P, 64), dtype=np.float32)
    x = np.asarray(x, np.float32)
    in_maps = []
    for n in range(B):
        xT = np.ascontiguousarray(
            x[n].T.astype(NPBF16).reshape(KB, P, S).transpose(1, 0, 2))
        in_maps.append({
            "xT": xT, "wq": wq, "wk": wk, "wv": wv,
            "bq": bq, "bk": bk, "bv": bv, "on32": on32, "bsel": bsel,
            "bself": bself, "onesT": onesT,
        })
    return in_maps


def kernel(**inputs) -> np.ndarray:
    in_maps = host_prep(**inputs)
    nc = build_nc()
    res = run_bass_kernel_spmd(
        nc, in_maps, core_ids=list(range(B)),
        trace=bool(int(os.environ.get("KERNEL_TRACE", "0"))),
    )
    _NC_CACHE["last_res"] = res
    if res.exec_time_ns is not None:
        print(f"HW exec time: {res.exec_time_ns} ns")
    # out is stored transposed ([D, S] per core); un-transpose on host
    out = np.stack([res.results[i]["out"].T for i in range(B)], axis=0)
    return np.ascontiguousarray(out).astype(np.float32)
